# revision 55
# baseline (speedup 1.0000x reference)
"""PokeTransformer Bass/Tile kernel for Trainium2, 8-core data-parallel over batch.

Strategy (per core = one batch element, fp32 residual stream, bf16 GEMMs):
  - Residual stream x kept token-major [480, 512] fp32 in SBUF (4x [120, 512]).
  - LayerNorm token-major: bn_stats/bn_aggr for mean/var on DVE, rsqrt via
    magic-seed + 2 Newton iterations (fp32-accurate, avoids ACT sqrt table
    loads); gamma/beta folded into the following weights on the host.
  - GEMM inputs bf16 (weights pre-cast on host; activations cast in the
    psum->sbuf copies), fp32 PSUM accumulation. Activations transposed to
    feature-major via PE transpose-mode matmuls (bf16, 1 cyc/row).
  - Block-sparse episodic attention (mask = same-turn 15-token blocks + up to
    n_prev_fields previous field tokens, episode-gated):
      * own-turn scoresT per head: 4 chunk-diagonal [120k x 120q] blocks
      * field scoresT per head: [32 field keys x 480 q] dense
      * softcap+softmax numerator: exp(CAP*tanh(s/(8*CAP))) via two ACT
        passes (scale folded into ACT), then 0/1 mask multiply
        (host-precomputed masks); no max-subtraction needed (|logit|<=CAP)
      * AV token-major with a ones-augmented V column producing the softmax
        denominators for free in the same PSUM accumulation
  - Biases applied via per-partition ACT/DVE bias operands (feature-major
    outputs) or K=1 ones-row matmuls (token-major outputs).
  - Per-layer weight streaming with double-buffered pools; layer-0 x/Wqkv
    DMAs hoisted ahead of constants.
"""

import numpy as np

L, D, H, HD, FF = 4, 512, 8, 64, 1024
B, S = 8, 480
TPT = 15
NCH, CK = 4, 120          # 4 chunks of 8 turns = 120 tokens
NKT = D // 128            # 4 feature k-tiles
NFT = FF // 128           # 8 ff tiles
CAP = 6.0
EPS = 1e-5
NEG_HALF, THREE_HALF = -0.5, 1.5
MAGIC_P1 = 0x5F3759DF + 1
N_CORES = 8

_CACHE = {}


def _build_module():
    import concourse.bacc as bacc
    import concourse.mybir as mybir
    from concourse.tile import TileContext
    from concourse.masks import make_identity

    dt = mybir.dt
    f32, f32r, bf16, i32 = dt.float32, dt.float32r, dt.bfloat16, dt.int32
    AF = mybir.ActivationFunctionType
    ALU = mybir.AluOpType

    nc = bacc.Bacc("TRN2", target_bir_lowering=False, debug=False,
                   num_devices=N_CORES)

    # ---- DRAM I/O (per core) ----
    x_d = nc.dram_tensor("x", [S, D], f32, kind="ExternalInput")
    wqkv_d = nc.dram_tensor("wqkv", [L, D, 3 * D], bf16, kind="ExternalInput")
    wo_d = nc.dram_tensor("wo", [L, D, D], bf16, kind="ExternalInput")
    w1_d = nc.dram_tensor("w1", [L, D, FF], bf16, kind="ExternalInput")
    w2_d = nc.dram_tensor("w2", [L, FF, D], bf16, kind="ExternalInput")
    bqk_d = nc.dram_tensor("bqk", [L, 128, 8], f32, kind="ExternalInput")
    bf1_d = nc.dram_tensor("bf1", [L, 128, 8], f32, kind="ExternalInput")
    bv_d = nc.dram_tensor("bv", [L, 1, D], bf16, kind="ExternalInput")
    bo_d = nc.dram_tensor("bo", [L, 1, D], bf16, kind="ExternalInput")
    bf2_d = nc.dram_tensor("bf2", [L, 1, D], bf16, kind="ExternalInput")
    omask_d = nc.dram_tensor("own_maskT", [CK, S], bf16, kind="ExternalInput")
    fmask_d = nc.dram_tensor("field_maskT", [32, S], bf16, kind="ExternalInput")
    out_d = nc.dram_tensor("out", [S, D], f32, kind="ExternalOutput")

    from contextlib import ExitStack
    with TileContext(nc) as tc, ExitStack() as ctx:
        const = ctx.enter_context(tc.tile_pool(name="const", bufs=1))
        acts = ctx.enter_context(tc.tile_pool(name="acts", bufs=1))
        wq_pool = ctx.enter_context(tc.tile_pool(name="wq", bufs=2))
        wo_pool = ctx.enter_context(tc.tile_pool(name="wop", bufs=2))
        w1_pool = ctx.enter_context(tc.tile_pool(name="w1p", bufs=2))
        w2_pool = ctx.enter_context(tc.tile_pool(name="w2p", bufs=2))
        pp_tp = ctx.enter_context(tc.tile_pool(name="pp_tp", bufs=2, space="PSUM"))
        pp_mm = ctx.enter_context(tc.tile_pool(name="pp_mm", bufs=2, space="PSUM"))
        pp_sc = ctx.enter_context(tc.tile_pool(name="pp_sc", bufs=2, space="PSUM"))
        pp_av = ctx.enter_context(tc.tile_pool(name="pp_av", bufs=1, space="PSUM"))

        # ---- input + first-layer weights first (startup critical path) ----
        x_tok = [acts.tile([CK, D], f32, tag=f"x{c}", name=f"x_tok{c}")
                 for c in range(NCH)]
        for c in range(NCH):
            nc.sync.dma_start(out=x_tok[c], in_=x_d[CK * c:CK * (c + 1), :])

        def load_wq(lidx):
            wq_sb = wq_pool.tile([128, NKT, 3 * D], bf16, tag="wq",
                                 name=f"wq{lidx}")
            for kt in range(NKT):
                nc.sync.dma_start(out=wq_sb[:, kt, :],
                                  in_=wqkv_d[lidx, 128 * kt:128 * (kt + 1), :])
            return wq_sb

        wq_next = load_wq(0)

        # ---- constants ----
        ident = const.tile([128, 128], bf16)
        make_identity(nc, ident)
        ones_row = const.tile([1, 128], bf16)
        nc.vector.memset(ones_row, 1.0)
        omask = const.tile([CK, S], bf16)
        nc.sync.dma_start(out=omask, in_=omask_d[:, :])
        fmask4 = const.tile([128, S], bf16)
        for g in range(4):
            nc.sync.dma_start(out=fmask4[32 * g:32 * (g + 1), :],
                              in_=fmask_d[:, :])
        bqk_sb = const.tile([128, L, 8], f32)
        nc.sync.dma_start(out=bqk_sb, in_=bqk_d.rearrange("l p m -> p l m"))
        bf1_sb = const.tile([128, L, 8], f32)
        nc.sync.dma_start(out=bf1_sb, in_=bf1_d.rearrange("l p m -> p l m"))
        bv_sb = const.tile([1, L, D], bf16)
        nc.sync.dma_start(out=bv_sb, in_=bv_d.rearrange("l one d -> one l d"))
        bo_sb = const.tile([1, L, D], bf16)
        nc.sync.dma_start(out=bo_sb, in_=bo_d.rearrange("l one d -> one l d"))
        bf2_sb = const.tile([1, L, D], bf16)
        nc.sync.dma_start(out=bf2_sb, in_=bf2_d.rearrange("l one d -> one l d"))

        # ---- persistent activations ----
        xn_fm = [acts.tile([128, S], bf16, tag=f"xnfm{f}", name=f"xn_fm{f}")
                 for f in range(NKT)]
        qk = [acts.tile([128, S], bf16, tag=f"qk{m}", name=f"qk{m}")
              for m in range(8)]
        vaug = [acts.tile([CK, H, 65], bf16, tag=f"vaug{c}", name=f"vaug{c}")
                for c in range(NCH)]
        for c in range(NCH):
            nc.vector.memset(vaug[c][:, :, 64:65], 1.0)
        vfield = acts.tile([128, H, 65], bf16)
        pf_sb = [acts.tile([128, S], bf16, tag=f"pf{g}", name=f"pf{g}")
                 for g in range(2)]
        attn_tok = [acts.tile([CK, D], bf16, tag=f"at{c}", name=f"attn_tok{c}")
                    for c in range(NCH)]
        attn_fm = [acts.tile([128, S], bf16, tag=f"afm{f}", name=f"attn_fm{f}")
                   for f in range(NKT)]
        ff_fm = [acts.tile([128, S], bf16, tag=f"ff{m}", name=f"ff_fm{m}")
                 for m in range(NFT)]
        pown = [acts.tile([CK, S], bf16, tag=f"pw{h}", name=f"pown{h}")
                for h in range(H)]

        small = ctx.enter_context(tc.tile_pool(name="small", bufs=2))
        xn_pool = ctx.enter_context(tc.tile_pool(name="xnp", bufs=4))

        def residual_with_stats(c, ps_in, sums, tag):
            """x_tok[c] += psum, fused with per-token sum(x) on DVE; sum(x^2)
            via an ACT Square pass with free accumulation. Feeds the next
            layer_norm without a separate bn_stats chain."""
            nc.vector.tensor_tensor_reduce(
                out=x_tok[c], in0=x_tok[c], in1=ps_in, scale=1.0, scalar=0.0,
                op0=ALU.add, op1=ALU.add, accum_out=sums[:, c, 0:1])
            sq = small.tile([CK, D], f32, tag="sqs", name=f"sq_{tag}{c}")
            nc.vector.tensor_tensor_reduce(
                out=sq, in0=x_tok[c], in1=x_tok[c], scale=1.0, scalar=0.0,
                op0=ALU.mult, op1=ALU.add, accum_out=sums[:, c, 1:2])

        def layer_norm(lidx, tag, sums=None):
            """Returns 4 normalized token-major tiles [CK, D].

            sums: optional [CK, NCH, 2] (sum(x), sum(x^2)) per chunk from
            residual_with_stats; None -> compute stats via bn_stats here.
            """
            veps = small.tile([CK, NCH, 1], f32, tag="veps", name=f"veps_{tag}")
            y = small.tile([CK, NCH, 1], f32, tag="nwt_y", name=f"y_{tag}")
            t1 = small.tile([CK, NCH, 1], f32, tag="nwt_t", name=f"t1_{tag}")
            nmr = small.tile([CK, NCH, 1], f32, tag="nmr", name=f"nmr_{tag}")
            mu = small.tile([CK, NCH, 1], f32, tag="mu", name=f"mu_{tag}")
            if sums is None:
                mv = small.tile([CK, NCH, 2], f32, tag="mv", name=f"mv_{tag}")
                for c in range(NCH):
                    bns = small.tile([CK, 6], f32, tag="bns",
                                     name=f"bns_{tag}{c}")
                    nc.vector.bn_stats(out=bns, in_=x_tok[c])
                    nc.vector.bn_aggr(out=mv[:, c, :], in_=bns)
                nc.vector.tensor_copy(out=mu, in_=mv[:, :, 0:1])
                nc.vector.tensor_scalar(out=veps, in0=mv[:, :, 1:2],
                                        scalar1=EPS, scalar2=None, op0=ALU.add)
            else:
                # mu = sx/n; veps = sxx/n + eps - mu^2
                nc.vector.tensor_scalar(out=mu, in0=sums[:, :, 0:1],
                                        scalar1=1.0 / D, scalar2=None,
                                        op0=ALU.mult)
                nc.vector.tensor_scalar(out=veps, in0=sums[:, :, 1:2],
                                        scalar1=1.0 / D, scalar2=EPS,
                                        op0=ALU.mult, op1=ALU.add)
                nc.vector.tensor_mul(t1, mu, mu)
                nc.vector.tensor_sub(veps, veps, t1)
            # rsqrt: magic-seed (int arithmetic done in f32, exact enough)
            # + 3 Newton iterations -> fp32-accurate
            nc.vector.tensor_copy(out=t1, in_=veps.bitcast(i32))
            nc.vector.tensor_scalar(out=t1, in0=t1, scalar1=-0.5,
                                    scalar2=float(0x5F3759DF),
                                    op0=ALU.mult, op1=ALU.add)
            nc.vector.tensor_copy(out=y.bitcast(i32), in_=t1)
            for _ in range(2):
                nc.vector.tensor_mul(t1, y, y)
                nc.vector.tensor_mul(t1, t1, veps)
                nc.vector.tensor_scalar(out=t1, in0=t1, scalar1=NEG_HALF,
                                        scalar2=THREE_HALF, op0=ALU.mult,
                                        op1=ALU.add)
                nc.vector.tensor_mul(y, y, t1)
            nc.vector.scalar_tensor_tensor(out=nmr, in0=mu,
                                           scalar=-1.0, in1=y,
                                           op0=ALU.mult, op1=ALU.mult)
            xn = []
            for c in range(NCH):
                xt = xn_pool.tile([CK, D], bf16, tag="xn", name=f"xn_{tag}{c}")
                nc.vector.tensor_scalar(out=xt, in0=x_tok[c],
                                        scalar1=y[:, c, :],
                                        scalar2=nmr[:, c, :],
                                        op0=ALU.mult, op1=ALU.add)
                xn.append(xt)
            return xn

        def transpose_to_fm(xn, dst, tag, by_chunk=False):
            """xn: 4 token-major [CK, D] tiles -> dst: 4 fm [128, S] tiles."""
            if by_chunk:
                # chunk-major with per-chunk copies: downstream per-chunk
                # consumers (Wo) can start after chunk 0 lands
                for c in range(NCH):
                    ps = pp_tp.tile([128, NKT, CK], bf16, tag="tp",
                                    name=f"tp_{tag}{c}")
                    sl = slice(CK * c, CK * (c + 1))
                    for f in range(NKT):
                        nc.tensor.transpose(ps[:, f, :],
                                            xn[c][:, 128 * f:128 * (f + 1)],
                                            ident[:CK, :CK])
                        if f % 2 == 0:
                            nc.scalar.copy(out=dst[f][:, sl], in_=ps[:, f, :])
                        else:
                            nc.vector.tensor_copy(out=dst[f][:, sl],
                                                  in_=ps[:, f, :])
                return
            for f in range(NKT):
                ps = pp_tp.tile([128, S], bf16, tag="tp", name=f"tp_{tag}{f}")
                for c in range(NCH):
                    nc.tensor.transpose(ps[:, CK * c:CK * (c + 1)],
                                        xn[c][:, 128 * f:128 * (f + 1)],
                                        ident[:CK, :CK])
                if f % 2 == 0:
                    nc.scalar.copy(out=dst[f], in_=ps)
                else:
                    nc.vector.tensor_copy(out=dst[f], in_=ps)

        for lidx in range(L):
            wq_sb = wq_next

            # ---- LN1 + transpose ----
            xn1 = layer_norm(lidx, f"a{lidx}")
            transpose_to_fm(xn1, xn_fm, f"a{lidx}")

            # ---- QKV (feature-major Q,K) ----
            # interleave q/k tiles so early heads' scores can start while
            # later qkv m-tiles still run
            for mt in (0, 4, 1, 5, 2, 6, 3, 7):
                ps = pp_mm.tile([128, 512], f32, tag="mm", name=f"qkv{lidx}_{mt}")
                for kt in range(NKT):
                    nc.tensor.matmul(ps[:, :S],
                                     wq_sb[:, kt, 128 * mt:128 * (mt + 1)],
                                     xn_fm[kt],
                                     start=(kt == 0), stop=(kt == NKT - 1))
                bias = bqk_sb[:, lidx, mt:mt + 1]
                if mt % 2 == 0:
                    nc.scalar.activation(out=qk[mt], in_=ps[:, :S],
                                         func=AF.Identity, bias=bias)
                else:
                    nc.vector.tensor_scalar(out=qk[mt], in0=ps[:, :S],
                                            scalar1=bias, scalar2=None,
                                            op0=ALU.add)

            # ---- this layer's remaining weights + next layer prefetch ----
            wo_sb = wo_pool.tile([128, NKT, D], bf16, tag="wo", name=f"wo{lidx}")
            for kt in range(NKT):
                nc.sync.dma_start(out=wo_sb[:, kt, :],
                                  in_=wo_d[lidx, 128 * kt:128 * (kt + 1), :])
            w1_sb = w1_pool.tile([128, NKT, FF], bf16, tag="w1", name=f"w1{lidx}")
            for kt in range(NKT):
                nc.sync.dma_start(out=w1_sb[:, kt, :],
                                  in_=w1_d[lidx, 128 * kt:128 * (kt + 1), :])
            w2_sb = w2_pool.tile([128, NFT, D], bf16, tag="w2", name=f"w2{lidx}")
            for kt in range(NFT):
                nc.sync.dma_start(out=w2_sb[:, kt, :],
                                  in_=w2_d[lidx, 128 * kt:128 * (kt + 1), :])
            if lidx + 1 < L:
                wq_next = load_wq(lidx + 1)

            # ---- V token-major (with bias via ones-row) ----
            for c in range(NCH):
                ps = pp_mm.tile([128, 512], f32, tag="mm", name=f"v{lidx}_{c}")
                for kt in range(NKT):
                    nc.tensor.matmul(ps[:CK, :],
                                     xn_fm[kt][:, CK * c:CK * (c + 1)],
                                     wq_sb[:, kt, 2 * D:3 * D],
                                     start=(kt == 0), stop=False)
                nc.tensor.matmul(ps[:CK, :], ones_row[:, :CK],
                                 bv_sb[:, lidx, :], start=False, stop=True)
                nc.vector.tensor_copy(
                    out=vaug[c][:, :, 0:64],
                    in_=ps[:CK, :].rearrange("p (h e) -> p h e", e=64))
            # gather field-token rows of V_aug (stride 15 partitions), then
            # replicate to all four 32-partition groups (matmul operands must
            # share a base partition with the p tiles they pair with)
            for c in range(NCH):
                nc.sync.dma_start(
                    out=vfield[8 * c:8 * (c + 1), :, :],
                    in_=vaug[c].rearrange("(t i) h e -> t i h e", i=TPT)[:, 0])
            for rep in range(1, 4):
                nc.sync.dma_start(out=vfield[32 * rep:32 * (rep + 1), :, :],
                                  in_=vfield[0:32, :, :])

            # ---- attention scores ----
            # field scores [32 fields x S] per head, 4 heads stacked per tile
            for g in range(2):
                ps = pp_sc.tile([128, S], f32, tag="sc", name=f"fs{lidx}_{g}")
                for j in range(4):
                    h = 4 * g + j
                    kt_, r0 = 4 + h // 2, 64 * (h % 2)
                    nc.tensor.matmul(
                        ps[32 * j:32 * (j + 1), :],
                        qk[kt_][r0:r0 + 64, :].rearrange(
                            "d (t i) -> d t i", i=TPT)[:, :, 0],
                        qk[h // 2][r0:r0 + 64, :],
                        tile_position=(r0, 32 * j))
                nc.scalar.activation(out=ps, in_=ps, func=AF.Tanh,
                                     scale=1.0 / (8.0 * CAP))
                nc.scalar.activation(out=pf_sb[g], in_=ps, func=AF.Exp,
                                     scale=CAP)
                nc.gpsimd.tensor_mul(pf_sb[g], pf_sb[g], fmask4)

            # own-turn scores per head: 4 chunk-diagonal [120x120] blocks
            for h in range(H):
                kt_, r0 = h // 2, 64 * (h % 2)
                ps = pp_sc.tile([128, S], f32, tag="sc", name=f"os{lidx}_{h}")
                for c in range(NCH):
                    sl = slice(CK * c, CK * (c + 1))
                    nc.tensor.matmul(ps[:CK, sl], qk[4 + kt_][r0:r0 + 64, sl],
                                     qk[kt_][r0:r0 + 64, sl])
                nc.scalar.activation(out=ps[:CK, :], in_=ps[:CK, :],
                                     func=AF.Tanh, scale=1.0 / (8.0 * CAP))
                nc.scalar.activation(out=pown[h], in_=ps[:CK, :], func=AF.Exp,
                                     scale=CAP)
                nc.vector.tensor_mul(pown[h], pown[h], omask)

            # ---- AV + denominators, normalize into attn_tok ----
            for c in range(NCH):
                ps = pp_av.tile([CK, 2, 512], f32, tag="av", name=f"av{lidx}_{c}")
                sl = slice(CK * c, CK * (c + 1))
                for h in range(H):
                    dst = ps[:, h // 4, 65 * (h % 4):65 * (h % 4) + 65]
                    nc.tensor.matmul(dst, pown[h][:, sl], vaug[c][:, h, :],
                                     start=True, stop=False)
                    nc.tensor.matmul(dst,
                                     pf_sb[h // 4][32 * (h % 4):32 * (h % 4) + 32, sl],
                                     vfield[32 * (h % 4):32 * (h % 4) + 32, h, :],
                                     start=False, stop=True,
                                     tile_position=(32 * (h % 4), 0))
                dnr = small.tile([CK, 2, 4, 1], f32, tag="dnr",
                                 name=f"dnr{lidx}_{c}")
                nc.vector.reciprocal(
                    out=dnr,
                    in_=ps[:, :, 0:260].rearrange(
                        "p g (h e) -> p g h e", e=65)[:, :, :, 64:65])
                # one bulk psum->sbuf copy, then cheap sbuf-only per-head
                # scales (avoids the psum access cost on each small op)
                au = small.tile([CK, H, 64], bf16, tag="au",
                                name=f"au{lidx}_{c}")
                nc.scalar.copy(
                    out=au.rearrange("p (g h) e -> p g h e", g=2),
                    in_=ps[:, :, 0:260].rearrange(
                        "p g (h e) -> p g h e", e=65)[:, :, :, 0:64])
                for h in range(H):
                    nc.vector.tensor_scalar(
                        out=attn_tok[c][:, 64 * h:64 * (h + 1)],
                        in0=au[:, h, :], scalar1=dnr[:, h // 4, h % 4, :],
                        scalar2=None, op0=ALU.mult)

            transpose_to_fm(attn_tok, attn_fm, f"o{lidx}", by_chunk=True)

            # ---- Wo + residual (fused with LN2 stats) ----
            for c in range(NCH):
                ps = pp_mm.tile([128, 512], f32, tag="mm", name=f"woo{lidx}_{c}")
                sl = slice(CK * c, CK * (c + 1))
                for kt in range(NKT):
                    nc.tensor.matmul(ps[:CK, :], attn_fm[kt][:, sl],
                                     wo_sb[:, kt, :],
                                     start=(kt == 0), stop=False)
                nc.tensor.matmul(ps[:CK, :], ones_row[:, :CK],
                                 bo_sb[:, lidx, :], start=False, stop=True)
                nc.vector.tensor_add(x_tok[c], x_tok[c], ps[:CK, :])

            # ---- LN2 + FFN ----
            xn2 = layer_norm(lidx, f"b{lidx}")
            transpose_to_fm(xn2, xn_fm, f"b{lidx}")

            for mt in range(NFT):
                ps = pp_mm.tile([128, 512], f32, tag="mm", name=f"ffm{lidx}_{mt}")
                for kt in range(NKT):
                    nc.tensor.matmul(ps[:, :S],
                                     w1_sb[:, kt, 128 * mt:128 * (mt + 1)],
                                     xn_fm[kt],
                                     start=(kt == 0), stop=(kt == NKT - 1))
                nc.scalar.activation(out=ff_fm[mt], in_=ps[:, :S], func=AF.Gelu,
                                     bias=bf1_sb[:, lidx, mt:mt + 1])

            for c in range(NCH):
                ps = pp_mm.tile([128, 512], f32, tag="mm", name=f"w2o{lidx}_{c}")
                sl = slice(CK * c, CK * (c + 1))
                for kt in range(NFT):
                    nc.tensor.matmul(ps[:CK, :], ff_fm[kt][:, sl],
                                     w2_sb[:, kt, :],
                                     start=(kt == 0), stop=False)
                nc.tensor.matmul(ps[:CK, :], ones_row[:, :CK],
                                 bf2_sb[:, lidx, :], start=False, stop=True)
                nc.vector.tensor_add(x_tok[c], x_tok[c], ps[:CK, :])

        for c in range(NCH):
            nc.sync.dma_start(out=out_d[CK * c:CK * (c + 1), :], in_=x_tok[c])

    nc.compile()
    return nc


def get_module():
    if "nc" not in _CACHE:
        _CACHE["nc"] = _build_module()
    return _CACHE["nc"]


def _episodic_masks(expanded_ids, n_prev):
    """Host-precomputed 0/1 masks in scoresT layouts.

    own_maskT[b][kk, 120c+qq]: k=120c+kk, q=120c+qq, same turn & same episode.
    field_maskT[b][t, q]: field key of turn t visible to query q.
    """
    eid = np.asarray(expanded_ids)
    idx = np.arange(S)
    turn = idx // TPT
    own = np.zeros((B, CK, S), np.float32)
    fld = np.zeros((B, 32, S), np.float32)
    for b in range(B):
        same_ep = eid[b][:, None] == eid[b][None, :]          # [k, q]
        same_turn = turn[:, None] == turn[None, :]
        full_own = (same_ep & same_turn)
        for c in range(NCH):
            sl = slice(CK * c, CK * (c + 1))
            own[b, :, sl] = full_own[sl, sl].astype(np.float32)
        kt = np.arange(32)
        qt = turn[None, :]                                     # [1, q]
        win = (kt[:, None] < qt) & (qt - kt[:, None] <= n_prev)
        ep = eid[b][TPT * kt][:, None] == eid[b][None, :]
        fld[b] = (win & ep).astype(np.float32)
    return own, fld


def _prep_inputs(x, expanded_ids, n_prev_fields, Wqkv, bqkv, Wo, bo, g1, b1,
                 g2, b2, W1, bf1, W2, bf2):
    f = np.float32
    x = np.ascontiguousarray(np.asarray(x, f))
    g1, b1 = np.asarray(g1, f), np.asarray(b1, f)
    g2, b2 = np.asarray(g2, f), np.asarray(b2, f)
    Wqkv, bqkv = np.asarray(Wqkv, f), np.asarray(bqkv, f)
    Wo, bo = np.asarray(Wo, f), np.asarray(bo, f)
    W1, bf1 = np.asarray(W1, f), np.asarray(bf1, f)
    W2, bf2 = np.asarray(W2, f), np.asarray(bf2, f)

    wqkv_eff = np.ascontiguousarray(g1[:, :, None] * Wqkv)
    bqkv_eff = bqkv + np.einsum("ld,ldm->lm", b1, Wqkv)
    w1_eff = np.ascontiguousarray(g2[:, :, None] * W1)
    bf1_eff = bf1 + np.einsum("ld,ldm->lm", b2, W1)

    import ml_dtypes
    bf = ml_dtypes.bfloat16

    bqk = np.ascontiguousarray(
        bqkv_eff[:, :2 * D].reshape(L, 8, 128).transpose(0, 2, 1))
    bf1h = np.ascontiguousarray(
        bf1_eff.reshape(L, 8, 128).transpose(0, 2, 1))
    bv = np.ascontiguousarray(bqkv_eff[:, 2 * D:].reshape(L, 1, D), bf)
    boh = np.ascontiguousarray(bo.reshape(L, 1, D), bf)
    bf2h = np.ascontiguousarray(bf2.reshape(L, 1, D), bf)

    own, fld = _episodic_masks(expanded_ids, int(n_prev_fields))

    shared = {
        "wqkv": np.ascontiguousarray(wqkv_eff, bf),
        "wo": np.ascontiguousarray(Wo, bf),
        "w1": np.ascontiguousarray(w1_eff, bf),
        "w2": np.ascontiguousarray(W2, bf),
        "bqk": bqk, "bf1": bf1h, "bv": bv, "bo": boh, "bf2": bf2h,
    }
    in_maps = []
    for b in range(B):
        m = dict(shared)
        m["x"] = np.ascontiguousarray(x[b])
        m["own_maskT"] = np.ascontiguousarray(own[b], bf)
        m["field_maskT"] = np.ascontiguousarray(fld[b], bf)
        in_maps.append(m)
    return in_maps


def kernel(**inputs) -> np.ndarray:
    from concourse.bass_utils import run_bass_kernel_spmd

    nc = get_module()
    in_maps = _prep_inputs(**inputs)
    res = run_bass_kernel_spmd(nc, in_maps, list(range(N_CORES)))
    out = np.stack([res.results[b]["out"] for b in range(B)], axis=0)
    return out.astype(np.float32)


# revision 57
# speedup vs baseline: 1.0241x; 1.0241x over previous
"""PokeTransformer Bass/Tile kernel for Trainium2, 8-core data-parallel over batch.

Strategy (per core = one batch element, fp32 residual stream, bf16 GEMMs):
  - Residual stream x kept token-major [480, 512] fp32 in SBUF (4x [120, 512]).
  - LayerNorm token-major: bn_stats/bn_aggr for mean/var on DVE, rsqrt via
    magic-seed + 2 Newton iterations (fp32-accurate, avoids ACT sqrt table
    loads); gamma/beta folded into the following weights on the host.
  - GEMM inputs bf16 (weights pre-cast on host; activations cast in the
    psum->sbuf copies), fp32 PSUM accumulation. Activations transposed to
    feature-major via PE transpose-mode matmuls (bf16, 1 cyc/row).
  - Block-sparse episodic attention (mask = same-turn 15-token blocks + up to
    n_prev_fields previous field tokens, episode-gated):
      * own-turn scoresT per head: 4 chunk-diagonal [120k x 120q] blocks
      * field scoresT per head: [32 field keys x 480 q] dense
      * softcap+softmax numerator: exp(CAP*tanh(s/(8*CAP))) via two ACT
        passes (scale folded into ACT), then 0/1 mask multiply
        (host-precomputed masks); no max-subtraction needed (|logit|<=CAP)
      * AV token-major with a ones-augmented V column producing the softmax
        denominators for free in the same PSUM accumulation
  - Biases applied via per-partition ACT/DVE bias operands (feature-major
    outputs) or K=1 ones-row matmuls (token-major outputs).
  - Per-layer weight streaming with double-buffered pools; layer-0 x/Wqkv
    DMAs hoisted ahead of constants.
"""

import numpy as np

L, D, H, HD, FF = 4, 512, 8, 64, 1024
B, S = 8, 480
TPT = 15
NCH, CK = 4, 120          # 4 chunks of 8 turns = 120 tokens
NKT = D // 128            # 4 feature k-tiles
NFT = FF // 128           # 8 ff tiles
CAP = 6.0
EPS = 1e-5
NEG_HALF, THREE_HALF = -0.5, 1.5
MAGIC_P1 = 0x5F3759DF + 1
N_CORES = 8

_CACHE = {}


def _build_module():
    import concourse.bacc as bacc
    import concourse.mybir as mybir
    from concourse.tile import TileContext
    from concourse.masks import make_identity

    dt = mybir.dt
    f32, f32r, bf16, i32 = dt.float32, dt.float32r, dt.bfloat16, dt.int32
    AF = mybir.ActivationFunctionType
    ALU = mybir.AluOpType

    nc = bacc.Bacc("TRN2", target_bir_lowering=False, debug=False,
                   num_devices=N_CORES)

    # ---- DRAM I/O (per core) ----
    x_d = nc.dram_tensor("x", [S, D], f32, kind="ExternalInput")
    wqkv_d = nc.dram_tensor("wqkv", [L, D, 3 * D], bf16, kind="ExternalInput")
    wo_d = nc.dram_tensor("wo", [L, D, D], bf16, kind="ExternalInput")
    w1_d = nc.dram_tensor("w1", [L, D, FF], bf16, kind="ExternalInput")
    w2_d = nc.dram_tensor("w2", [L, FF, D], bf16, kind="ExternalInput")
    bqk_d = nc.dram_tensor("bqk", [L, 128, 8], f32, kind="ExternalInput")
    bf1_d = nc.dram_tensor("bf1", [L, 128, 8], f32, kind="ExternalInput")
    bv_d = nc.dram_tensor("bv", [L, 1, D], bf16, kind="ExternalInput")
    bo_d = nc.dram_tensor("bo", [L, 1, D], bf16, kind="ExternalInput")
    bf2_d = nc.dram_tensor("bf2", [L, 1, D], bf16, kind="ExternalInput")
    omask_d = nc.dram_tensor("own_maskT", [CK, S], bf16, kind="ExternalInput")
    fmask_d = nc.dram_tensor("field_maskT", [32, S], bf16, kind="ExternalInput")
    out_d = nc.dram_tensor("out", [S, D], f32, kind="ExternalOutput")

    from contextlib import ExitStack
    with TileContext(nc) as tc, ExitStack() as ctx:
        const = ctx.enter_context(tc.tile_pool(name="const", bufs=1))
        acts = ctx.enter_context(tc.tile_pool(name="acts", bufs=1))
        wq_pool = ctx.enter_context(tc.tile_pool(name="wq", bufs=2))
        wo_pool = ctx.enter_context(tc.tile_pool(name="wop", bufs=2))
        w1_pool = ctx.enter_context(tc.tile_pool(name="w1p", bufs=2))
        w2_pool = ctx.enter_context(tc.tile_pool(name="w2p", bufs=2))
        pp_tp = ctx.enter_context(tc.tile_pool(name="pp_tp", bufs=2, space="PSUM"))
        pp_mm = ctx.enter_context(tc.tile_pool(name="pp_mm", bufs=2, space="PSUM"))
        pp_sc = ctx.enter_context(tc.tile_pool(name="pp_sc", bufs=2, space="PSUM"))
        pp_av = ctx.enter_context(tc.tile_pool(name="pp_av", bufs=1, space="PSUM"))

        # ---- input + first-layer weights first (startup critical path) ----
        x_tok = [acts.tile([CK, D], f32, tag=f"x{c}", name=f"x_tok{c}")
                 for c in range(NCH)]
        for c in range(NCH):
            nc.sync.dma_start(out=x_tok[c], in_=x_d[CK * c:CK * (c + 1), :])

        def load_wq(lidx):
            wq_sb = wq_pool.tile([128, NKT, 3 * D], bf16, tag="wq",
                                 name=f"wq{lidx}")
            for kt in range(NKT):
                nc.sync.dma_start(out=wq_sb[:, kt, :],
                                  in_=wqkv_d[lidx, 128 * kt:128 * (kt + 1), :])
            return wq_sb

        wq_next = load_wq(0)

        # ---- constants ----
        ident = const.tile([128, 128], bf16)
        make_identity(nc, ident)
        ones_row = const.tile([1, 128], bf16)
        nc.vector.memset(ones_row, 1.0)
        omask = const.tile([CK, S], bf16)
        nc.sync.dma_start(out=omask, in_=omask_d[:, :])
        fmask4 = const.tile([128, S], bf16)
        for g in range(4):
            nc.sync.dma_start(out=fmask4[32 * g:32 * (g + 1), :],
                              in_=fmask_d[:, :])
        bqk_sb = const.tile([128, L, 8], f32)
        nc.sync.dma_start(out=bqk_sb, in_=bqk_d.rearrange("l p m -> p l m"))
        bf1_sb = const.tile([128, L, 8], f32)
        nc.sync.dma_start(out=bf1_sb, in_=bf1_d.rearrange("l p m -> p l m"))
        bv_sb = const.tile([1, L, D], bf16)
        nc.sync.dma_start(out=bv_sb, in_=bv_d.rearrange("l one d -> one l d"))
        bo_sb = const.tile([1, L, D], bf16)
        nc.sync.dma_start(out=bo_sb, in_=bo_d.rearrange("l one d -> one l d"))
        bf2_sb = const.tile([1, L, D], bf16)
        nc.sync.dma_start(out=bf2_sb, in_=bf2_d.rearrange("l one d -> one l d"))

        # ---- persistent activations ----
        xn_fm = [acts.tile([128, S], bf16, tag=f"xnfm{f}", name=f"xn_fm{f}")
                 for f in range(NKT)]
        qk = [acts.tile([128, S], bf16, tag=f"qk{m}", name=f"qk{m}")
              for m in range(8)]
        vaug = [acts.tile([CK, H, 65], bf16, tag=f"vaug{c}", name=f"vaug{c}")
                for c in range(NCH)]
        for c in range(NCH):
            nc.vector.memset(vaug[c][:, :, 64:65], 1.0)
        vfield = acts.tile([128, H, 65], bf16)
        pf_sb = [acts.tile([128, S], bf16, tag=f"pf{g}", name=f"pf{g}")
                 for g in range(2)]
        attn_tok = [acts.tile([CK, D], bf16, tag=f"at{c}", name=f"attn_tok{c}")
                    for c in range(NCH)]
        attn_fm = [acts.tile([128, S], bf16, tag=f"afm{f}", name=f"attn_fm{f}")
                   for f in range(NKT)]
        ff_fm = [acts.tile([128, S], bf16, tag=f"ff{m}", name=f"ff_fm{m}")
                 for m in range(NFT)]
        pown = [acts.tile([CK, S], bf16, tag=f"pw{h}", name=f"pown{h}")
                for h in range(H)]

        small = ctx.enter_context(tc.tile_pool(name="small", bufs=2))
        xn_pool = ctx.enter_context(tc.tile_pool(name="xnp", bufs=4))

        def residual_with_stats(c, ps_in, sums, tag):
            """x_tok[c] += psum, fused with per-token sum(x) on DVE; sum(x^2)
            via an ACT Square pass with free accumulation. Feeds the next
            layer_norm without a separate bn_stats chain."""
            nc.vector.tensor_tensor_reduce(
                out=x_tok[c], in0=x_tok[c], in1=ps_in, scale=1.0, scalar=0.0,
                op0=ALU.add, op1=ALU.add, accum_out=sums[:, c, 0:1])
            sq = small.tile([CK, D], f32, tag="sqs", name=f"sq_{tag}{c}")
            nc.vector.tensor_tensor_reduce(
                out=sq, in0=x_tok[c], in1=x_tok[c], scale=1.0, scalar=0.0,
                op0=ALU.mult, op1=ALU.add, accum_out=sums[:, c, 1:2])

        def layer_norm(lidx, tag, sums=None):
            """Returns 4 normalized token-major tiles [CK, D].

            sums: optional [CK, NCH, 2] (sum(x), sum(x^2)) per chunk from
            residual_with_stats; None -> compute stats via bn_stats here.
            """
            veps = small.tile([CK, NCH, 1], f32, tag="veps", name=f"veps_{tag}")
            y = small.tile([CK, NCH, 1], f32, tag="nwt_y", name=f"y_{tag}")
            t1 = small.tile([CK, NCH, 1], f32, tag="nwt_t", name=f"t1_{tag}")
            nmr = small.tile([CK, NCH, 1], f32, tag="nmr", name=f"nmr_{tag}")
            mv = small.tile([CK, NCH, 2], f32, tag="mv", name=f"mv_{tag}")
            xn = []
            # fully per-chunk: chunk c's normalize depends only on chunk c's
            # stats, so chunks 0-2 normalize while later chunks' stats (and
            # the producing GEMM) still run
            for c in range(NCH):
                bns = small.tile([CK, 6], f32, tag="bns",
                                 name=f"bns_{tag}{c}")
                nc.vector.bn_stats(out=bns, in_=x_tok[c])
                nc.vector.bn_aggr(out=mv[:, c, :], in_=bns)
                vc = veps[:, c, :]
                yc = y[:, c, :]
                tc_ = t1[:, c, :]
                nc.vector.tensor_scalar(out=vc, in0=mv[:, c, 1:2],
                                        scalar1=EPS, scalar2=None, op0=ALU.add)
                nc.vector.tensor_copy(out=tc_, in_=vc.bitcast(i32))
                nc.vector.tensor_scalar(out=tc_, in0=tc_, scalar1=-0.5,
                                        scalar2=float(0x5F3759DF),
                                        op0=ALU.mult, op1=ALU.add)
                nc.vector.tensor_copy(out=yc.bitcast(i32), in_=tc_)
                for _ in range(2):
                    nc.vector.tensor_mul(tc_, yc, yc)
                    nc.vector.tensor_mul(tc_, tc_, vc)
                    nc.vector.tensor_scalar(out=tc_, in0=tc_, scalar1=NEG_HALF,
                                            scalar2=THREE_HALF, op0=ALU.mult,
                                            op1=ALU.add)
                    nc.vector.tensor_mul(yc, yc, tc_)
                nc.vector.scalar_tensor_tensor(out=nmr[:, c, :],
                                               in0=mv[:, c, 0:1],
                                               scalar=-1.0, in1=yc,
                                               op0=ALU.mult, op1=ALU.mult)
                xt = xn_pool.tile([CK, D], bf16, tag="xn", name=f"xn_{tag}{c}")
                nc.vector.tensor_scalar(out=xt, in0=x_tok[c],
                                        scalar1=yc,
                                        scalar2=nmr[:, c, :],
                                        op0=ALU.mult, op1=ALU.add)
                xn.append(xt)
            return xn

        def transpose_to_fm(xn, dst, tag, by_chunk=False):
            """xn: 4 token-major [CK, D] tiles -> dst: 4 fm [128, S] tiles."""
            if by_chunk:
                # chunk-major with per-chunk copies: downstream per-chunk
                # consumers (Wo) can start after chunk 0 lands
                for c in range(NCH):
                    ps = pp_tp.tile([128, NKT, CK], bf16, tag="tp",
                                    name=f"tp_{tag}{c}")
                    sl = slice(CK * c, CK * (c + 1))
                    for f in range(NKT):
                        nc.tensor.transpose(ps[:, f, :],
                                            xn[c][:, 128 * f:128 * (f + 1)],
                                            ident[:CK, :CK])
                        if f % 2 == 0:
                            nc.scalar.copy(out=dst[f][:, sl], in_=ps[:, f, :])
                        else:
                            nc.vector.tensor_copy(out=dst[f][:, sl],
                                                  in_=ps[:, f, :])
                return
            for f in range(NKT):
                ps = pp_tp.tile([128, S], bf16, tag="tp", name=f"tp_{tag}{f}")
                for c in range(NCH):
                    nc.tensor.transpose(ps[:, CK * c:CK * (c + 1)],
                                        xn[c][:, 128 * f:128 * (f + 1)],
                                        ident[:CK, :CK])
                if f % 2 == 0:
                    nc.scalar.copy(out=dst[f], in_=ps)
                else:
                    nc.vector.tensor_copy(out=dst[f], in_=ps)

        for lidx in range(L):
            wq_sb = wq_next

            # ---- LN1 + transpose ----
            xn1 = layer_norm(lidx, f"a{lidx}")
            transpose_to_fm(xn1, xn_fm, f"a{lidx}")

            # ---- QKV (feature-major Q,K) ----
            # interleave q/k tiles so early heads' scores can start while
            # later qkv m-tiles still run
            for mt in (0, 4, 1, 5, 2, 6, 3, 7):
                ps = pp_mm.tile([128, 512], f32, tag="mm", name=f"qkv{lidx}_{mt}")
                for kt in range(NKT):
                    nc.tensor.matmul(ps[:, :S],
                                     wq_sb[:, kt, 128 * mt:128 * (mt + 1)],
                                     xn_fm[kt],
                                     start=(kt == 0), stop=(kt == NKT - 1))
                bias = bqk_sb[:, lidx, mt:mt + 1]
                if mt % 2 == 0:
                    nc.scalar.activation(out=qk[mt], in_=ps[:, :S],
                                         func=AF.Identity, bias=bias)
                else:
                    nc.vector.tensor_scalar(out=qk[mt], in0=ps[:, :S],
                                            scalar1=bias, scalar2=None,
                                            op0=ALU.add)

            # ---- this layer's remaining weights + next layer prefetch ----
            wo_sb = wo_pool.tile([128, NKT, D], bf16, tag="wo", name=f"wo{lidx}")
            for kt in range(NKT):
                nc.sync.dma_start(out=wo_sb[:, kt, :],
                                  in_=wo_d[lidx, 128 * kt:128 * (kt + 1), :])
            w1_sb = w1_pool.tile([128, NKT, FF], bf16, tag="w1", name=f"w1{lidx}")
            for kt in range(NKT):
                nc.sync.dma_start(out=w1_sb[:, kt, :],
                                  in_=w1_d[lidx, 128 * kt:128 * (kt + 1), :])
            w2_sb = w2_pool.tile([128, NFT, D], bf16, tag="w2", name=f"w2{lidx}")
            for kt in range(NFT):
                nc.sync.dma_start(out=w2_sb[:, kt, :],
                                  in_=w2_d[lidx, 128 * kt:128 * (kt + 1), :])
            if lidx + 1 < L:
                wq_next = load_wq(lidx + 1)

            # ---- V token-major (with bias via ones-row) ----
            for c in range(NCH):
                ps = pp_mm.tile([128, 512], f32, tag="mm", name=f"v{lidx}_{c}")
                for kt in range(NKT):
                    nc.tensor.matmul(ps[:CK, :],
                                     xn_fm[kt][:, CK * c:CK * (c + 1)],
                                     wq_sb[:, kt, 2 * D:3 * D],
                                     start=(kt == 0), stop=False)
                nc.tensor.matmul(ps[:CK, :], ones_row[:, :CK],
                                 bv_sb[:, lidx, :], start=False, stop=True)
                nc.vector.tensor_copy(
                    out=vaug[c][:, :, 0:64],
                    in_=ps[:CK, :].rearrange("p (h e) -> p h e", e=64))
            # gather field-token rows of V_aug (stride 15 partitions), then
            # replicate to all four 32-partition groups (matmul operands must
            # share a base partition with the p tiles they pair with)
            for c in range(NCH):
                nc.sync.dma_start(
                    out=vfield[8 * c:8 * (c + 1), :, :],
                    in_=vaug[c].rearrange("(t i) h e -> t i h e", i=TPT)[:, 0])
            for rep in range(1, 4):
                nc.sync.dma_start(out=vfield[32 * rep:32 * (rep + 1), :, :],
                                  in_=vfield[0:32, :, :])

            # ---- attention scores ----
            # field scores [32 fields x S] per head, 4 heads stacked per tile
            for g in range(2):
                ps = pp_sc.tile([128, S], f32, tag="sc", name=f"fs{lidx}_{g}")
                for j in range(4):
                    h = 4 * g + j
                    kt_, r0 = 4 + h // 2, 64 * (h % 2)
                    nc.tensor.matmul(
                        ps[32 * j:32 * (j + 1), :],
                        qk[kt_][r0:r0 + 64, :].rearrange(
                            "d (t i) -> d t i", i=TPT)[:, :, 0],
                        qk[h // 2][r0:r0 + 64, :],
                        tile_position=(r0, 32 * j))
                nc.scalar.activation(out=ps, in_=ps, func=AF.Tanh,
                                     scale=1.0 / (8.0 * CAP))
                nc.scalar.activation(out=pf_sb[g], in_=ps, func=AF.Exp,
                                     scale=CAP)
                nc.gpsimd.tensor_mul(pf_sb[g], pf_sb[g], fmask4)

            # own-turn scores per head: 4 chunk-diagonal [120x120] blocks
            for h in range(H):
                kt_, r0 = h // 2, 64 * (h % 2)
                ps = pp_sc.tile([128, S], f32, tag="sc", name=f"os{lidx}_{h}")
                for c in range(NCH):
                    sl = slice(CK * c, CK * (c + 1))
                    nc.tensor.matmul(ps[:CK, sl], qk[4 + kt_][r0:r0 + 64, sl],
                                     qk[kt_][r0:r0 + 64, sl])
                nc.scalar.activation(out=ps[:CK, :], in_=ps[:CK, :],
                                     func=AF.Tanh, scale=1.0 / (8.0 * CAP))
                nc.scalar.activation(out=pown[h], in_=ps[:CK, :], func=AF.Exp,
                                     scale=CAP)
                nc.vector.tensor_mul(pown[h], pown[h], omask)

            # ---- AV + denominators, normalize into attn_tok ----
            for c in range(NCH):
                ps = pp_av.tile([CK, 2, 512], f32, tag="av", name=f"av{lidx}_{c}")
                sl = slice(CK * c, CK * (c + 1))
                for h in range(H):
                    dst = ps[:, h // 4, 65 * (h % 4):65 * (h % 4) + 65]
                    nc.tensor.matmul(dst, pown[h][:, sl], vaug[c][:, h, :],
                                     start=True, stop=False)
                    nc.tensor.matmul(dst,
                                     pf_sb[h // 4][32 * (h % 4):32 * (h % 4) + 32, sl],
                                     vfield[32 * (h % 4):32 * (h % 4) + 32, h, :],
                                     start=False, stop=True,
                                     tile_position=(32 * (h % 4), 0))
                dnr = small.tile([CK, 2, 4, 1], f32, tag="dnr",
                                 name=f"dnr{lidx}_{c}")
                nc.vector.reciprocal(
                    out=dnr,
                    in_=ps[:, :, 0:260].rearrange(
                        "p g (h e) -> p g h e", e=65)[:, :, :, 64:65])
                # one bulk psum->sbuf copy, then cheap sbuf-only per-head
                # scales (avoids the psum access cost on each small op)
                au = small.tile([CK, H, 64], bf16, tag="au",
                                name=f"au{lidx}_{c}")
                nc.scalar.copy(
                    out=au.rearrange("p (g h) e -> p g h e", g=2),
                    in_=ps[:, :, 0:260].rearrange(
                        "p g (h e) -> p g h e", e=65)[:, :, :, 0:64])
                for h in range(H):
                    nc.vector.tensor_scalar(
                        out=attn_tok[c][:, 64 * h:64 * (h + 1)],
                        in0=au[:, h, :], scalar1=dnr[:, h // 4, h % 4, :],
                        scalar2=None, op0=ALU.mult)

            transpose_to_fm(attn_tok, attn_fm, f"o{lidx}", by_chunk=True)

            # ---- Wo + residual (fused with LN2 stats) ----
            for c in range(NCH):
                ps = pp_mm.tile([128, 512], f32, tag="mm", name=f"woo{lidx}_{c}")
                sl = slice(CK * c, CK * (c + 1))
                for kt in range(NKT):
                    nc.tensor.matmul(ps[:CK, :], attn_fm[kt][:, sl],
                                     wo_sb[:, kt, :],
                                     start=(kt == 0), stop=False)
                nc.tensor.matmul(ps[:CK, :], ones_row[:, :CK],
                                 bo_sb[:, lidx, :], start=False, stop=True)
                nc.vector.tensor_add(x_tok[c], x_tok[c], ps[:CK, :])

            # ---- LN2 + FFN ----
            xn2 = layer_norm(lidx, f"b{lidx}")
            transpose_to_fm(xn2, xn_fm, f"b{lidx}")

            for mt in range(NFT):
                ps = pp_mm.tile([128, 512], f32, tag="mm", name=f"ffm{lidx}_{mt}")
                for kt in range(NKT):
                    nc.tensor.matmul(ps[:, :S],
                                     w1_sb[:, kt, 128 * mt:128 * (mt + 1)],
                                     xn_fm[kt],
                                     start=(kt == 0), stop=(kt == NKT - 1))
                nc.scalar.activation(out=ff_fm[mt], in_=ps[:, :S], func=AF.Gelu,
                                     bias=bf1_sb[:, lidx, mt:mt + 1])

            for c in range(NCH):
                ps = pp_mm.tile([128, 512], f32, tag="mm", name=f"w2o{lidx}_{c}")
                sl = slice(CK * c, CK * (c + 1))
                for kt in range(NFT):
                    nc.tensor.matmul(ps[:CK, :], ff_fm[kt][:, sl],
                                     w2_sb[:, kt, :],
                                     start=(kt == 0), stop=False)
                nc.tensor.matmul(ps[:CK, :], ones_row[:, :CK],
                                 bf2_sb[:, lidx, :], start=False, stop=True)
                nc.vector.tensor_add(x_tok[c], x_tok[c], ps[:CK, :])

        for c in range(NCH):
            nc.sync.dma_start(out=out_d[CK * c:CK * (c + 1), :], in_=x_tok[c])

    nc.compile()
    return nc


def get_module():
    if "nc" not in _CACHE:
        _CACHE["nc"] = _build_module()
    return _CACHE["nc"]


def _episodic_masks(expanded_ids, n_prev):
    """Host-precomputed 0/1 masks in scoresT layouts.

    own_maskT[b][kk, 120c+qq]: k=120c+kk, q=120c+qq, same turn & same episode.
    field_maskT[b][t, q]: field key of turn t visible to query q.
    """
    eid = np.asarray(expanded_ids)
    idx = np.arange(S)
    turn = idx // TPT
    own = np.zeros((B, CK, S), np.float32)
    fld = np.zeros((B, 32, S), np.float32)
    for b in range(B):
        same_ep = eid[b][:, None] == eid[b][None, :]          # [k, q]
        same_turn = turn[:, None] == turn[None, :]
        full_own = (same_ep & same_turn)
        for c in range(NCH):
            sl = slice(CK * c, CK * (c + 1))
            own[b, :, sl] = full_own[sl, sl].astype(np.float32)
        kt = np.arange(32)
        qt = turn[None, :]                                     # [1, q]
        win = (kt[:, None] < qt) & (qt - kt[:, None] <= n_prev)
        ep = eid[b][TPT * kt][:, None] == eid[b][None, :]
        fld[b] = (win & ep).astype(np.float32)
    return own, fld


def _prep_inputs(x, expanded_ids, n_prev_fields, Wqkv, bqkv, Wo, bo, g1, b1,
                 g2, b2, W1, bf1, W2, bf2):
    f = np.float32
    x = np.ascontiguousarray(np.asarray(x, f))
    g1, b1 = np.asarray(g1, f), np.asarray(b1, f)
    g2, b2 = np.asarray(g2, f), np.asarray(b2, f)
    Wqkv, bqkv = np.asarray(Wqkv, f), np.asarray(bqkv, f)
    Wo, bo = np.asarray(Wo, f), np.asarray(bo, f)
    W1, bf1 = np.asarray(W1, f), np.asarray(bf1, f)
    W2, bf2 = np.asarray(W2, f), np.asarray(bf2, f)

    wqkv_eff = np.ascontiguousarray(g1[:, :, None] * Wqkv)
    bqkv_eff = bqkv + np.einsum("ld,ldm->lm", b1, Wqkv)
    w1_eff = np.ascontiguousarray(g2[:, :, None] * W1)
    bf1_eff = bf1 + np.einsum("ld,ldm->lm", b2, W1)

    import ml_dtypes
    bf = ml_dtypes.bfloat16

    bqk = np.ascontiguousarray(
        bqkv_eff[:, :2 * D].reshape(L, 8, 128).transpose(0, 2, 1))
    bf1h = np.ascontiguousarray(
        bf1_eff.reshape(L, 8, 128).transpose(0, 2, 1))
    bv = np.ascontiguousarray(bqkv_eff[:, 2 * D:].reshape(L, 1, D), bf)
    boh = np.ascontiguousarray(bo.reshape(L, 1, D), bf)
    bf2h = np.ascontiguousarray(bf2.reshape(L, 1, D), bf)

    own, fld = _episodic_masks(expanded_ids, int(n_prev_fields))

    shared = {
        "wqkv": np.ascontiguousarray(wqkv_eff, bf),
        "wo": np.ascontiguousarray(Wo, bf),
        "w1": np.ascontiguousarray(w1_eff, bf),
        "w2": np.ascontiguousarray(W2, bf),
        "bqk": bqk, "bf1": bf1h, "bv": bv, "bo": boh, "bf2": bf2h,
    }
    in_maps = []
    for b in range(B):
        m = dict(shared)
        m["x"] = np.ascontiguousarray(x[b])
        m["own_maskT"] = np.ascontiguousarray(own[b], bf)
        m["field_maskT"] = np.ascontiguousarray(fld[b], bf)
        in_maps.append(m)
    return in_maps


def kernel(**inputs) -> np.ndarray:
    from concourse.bass_utils import run_bass_kernel_spmd

    nc = get_module()
    in_maps = _prep_inputs(**inputs)
    res = run_bass_kernel_spmd(nc, in_maps, list(range(N_CORES)))
    out = np.stack([res.results[b]["out"] for b in range(B)], axis=0)
    return out.astype(np.float32)
